# revision 25
# baseline (speedup 1.0000x reference)
"""Block-Circulant-Matrix Linear kernel for Trainium2 (8 NeuronCores, SPMD).

Reference computation:
    W[r*64+i, q*64+j] = w[r, q, (i-j) % 64]        (dense 1024x1024 from w[16,16,64])
    y = x @ W.T                                    (x: [32768, 1024] f32)

Strategy (data-parallel, per sharding hint):
  - Shard x along tokens across 8 cores (4096 tokens each); replicate the
    weight.  The dense W.T (the reference materializes exactly this) is built
    once on the host from the 64 KB compressed w and passed in as a 2 MB bf16
    tensor -- cheaper to DMA than an on-device skewed expansion, and it gives
    the TensorE a fully contiguous moving operand.
  - Per core, y_tile = x_tile @ W.T in bf16 on TensorE:
      * x is cast f32->bf16 on ScalarE, transposed per 128-chunk on TensorE
        (bf16 transpose = 1 cycle/row), and the bf16 PSUM transpose tile is
        copied to SBUF by VectorE at 2x 2-byte rate.
      * per token tile, the 8 transposes (for tile t) are interleaved between
        the 16 matmuls (for tile t-2) so every LDWEIGHTS hides under the
        previous matmul's 512-column stream.
      * a short warm-up spin of dummy transposes keeps the PE HAM clock-gate
        at 8/8 (2.4 GHz) through the DMA ramp, so the first real matmuls do
        not pay the 1.2 GHz cold window.
  - y is stored to DRAM in bf16 (halves store traffic; ~2e-3 max rel err,
    well within the 2e-2 gate) and upcast to f32 on the host.
  - All DMAs use large contiguous descriptors; no gather anywhere.
"""

import numpy as np

N_CORES = 8
N_TOKENS = 32768
TOK_PER_CORE = N_TOKENS // N_CORES  # 4096
IN_CH = 1024
OUT_CH = 1024
BS = 64
R = OUT_CH // BS  # 16
Q = IN_CH // BS   # 16
KCH = IN_CH // 128  # 8 k-chunks of 128 partitions
N_WARMUP = 34      # dummy transposes to keep the HAM clock-gate warm
DEPTH = 2          # matmuls trail transposes by DEPTH token tiles
XB_LEAD = 3        # x-tile DMA prefetch distance

_CACHE = {}


def build_nc(tok_per_core=TOK_PER_CORE):
    from contextlib import ExitStack

    import concourse.mybir as mybir
    import concourse.tile as tile
    from concourse import bacc

    f32 = mybir.dt.float32
    bf16 = mybir.dt.bfloat16

    nc = bacc.Bacc("TRN2", target_bir_lowering=False, debug=False)
    x = nc.dram_tensor("x", [tok_per_core, IN_CH], f32, kind="ExternalInput").ap()
    wt = nc.dram_tensor("wt", [IN_CH, OUT_CH], bf16, kind="ExternalInput").ap()
    ident = nc.dram_tensor("ident", [128, 128], bf16, kind="ExternalInput").ap()
    y = nc.dram_tensor("y", [tok_per_core, OUT_CH], bf16, kind="ExternalOutput").ap()

    n = tok_per_core // 128  # token tiles

    with tile.TileContext(nc) as tc, ExitStack() as ctx:
        const_pool = ctx.enter_context(tc.tile_pool(name="const", bufs=1))
        wt_pool = ctx.enter_context(tc.tile_pool(name="wt", bufs=1))
        xb_pool = ctx.enter_context(tc.tile_pool(name="xb", bufs=6))
        xh_pool = ctx.enter_context(tc.tile_pool(name="xh", bufs=4))
        xt_sb_pool = ctx.enter_context(tc.tile_pool(name="xt_sb", bufs=4))
        y_sb_pool = ctx.enter_context(tc.tile_pool(name="y_sb", bufs=4))
        xt_ps_pool = ctx.enter_context(tc.tile_pool(name="xt_ps", bufs=2, space="PSUM"))
        y_ps_pool = ctx.enter_context(tc.tile_pool(name="y_ps", bufs=2, space="PSUM"))
        warm_pool = ctx.enter_context(tc.tile_pool(name="warm", bufs=1, space="PSUM"))

        # host-provided identity lands first on the scalar queue (~0.7us --
        # building it with gpsimd memset/affine_select costs ~7.5us of
        # startup latency before the warm-up can begin)
        identity = const_pool.tile([128, 128], bf16)
        nc.scalar.dma_start(identity, ident)

        # --- PE warm-up: dummy transposes trip the HAM SHORT window (~3.4us
        # of sustained activity) so the real matmuls start at 2.4 GHz.  The
        # scratch PSUM tile is never read.
        warm_ps = warm_pool.tile([128, 128], bf16, name="warm")
        for _ in range(N_WARMUP):
            nc.tensor.transpose(warm_ps, identity, identity)

        # --- dense W.T chunks: wt_all[p, c*1024 + o] = wt[c*128 + p, o] ---
        # Two batched HWDGE DMAs (4 chunks each) on the scalar queue, issued
        # before cast(0): the 3x632ns DGE issue overhead hides entirely
        # inside the ~4us wait for the first x tile, and HWDGE delivers wt
        # long before the first matmul needs chunk 7 (SWDGE is too slow and
        # starves the early accumulations, resetting the HAM clock-gate).
        import concourse.bass as bass

        wt_all = wt_pool.tile([128, KCH * OUT_CH], bf16, name="wt_all")
        half_ch = KCH // 2
        for g in range(2):
            dst = bass.AP(
                wt_all.tensor,
                wt_all.offset + g * half_ch * OUT_CH,
                [[wt_all[:, :].ap[0][0], 128], [1, half_ch * OUT_CH]],
            )
            src = bass.AP(
                wt.tensor,
                wt.offset + g * half_ch * 128 * OUT_CH,
                [[OUT_CH, 128], [128 * OUT_CH, half_ch], [1, OUT_CH]],
            )
            nc.scalar.dma_start(dst, src)
        wt_sb = [wt_all[:, c * OUT_CH : (c + 1) * OUT_CH] for c in range(KCH)]

        xbs, xhs, xts = {}, {}, {}

        def emit_xb(t):
            xb = xb_pool.tile([128, IN_CH], f32, name=f"xb_{t}", tag="xb")
            nc.sync.dma_start(xb, x[t * 128 : (t + 1) * 128, :])
            xbs[t] = xb

        def emit_cast(t):
            xh = xh_pool.tile([128, IN_CH], bf16, name=f"xh_{t}", tag="xh")
            nc.scalar.copy(xh, xbs.pop(t))
            xhs[t] = xh

        for t in range(min(XB_LEAD, n)):
            emit_xb(t)
        emit_cast(0)

        for s in range(n + DEPTH):
            if s + XB_LEAD < n:
                emit_xb(s + XB_LEAD)
            if s + 1 < n:
                emit_cast(s + 1)
            if s < n:
                xh = xhs.pop(s)
                xt_ps = xt_ps_pool.tile(
                    [128, IN_CH], bf16, name=f"xt_ps_{s}", tag="xt_ps"
                )
            if s >= DEPTH:
                xt = xts.pop(s - DEPTH)
                y_ps = y_ps_pool.tile(
                    [128, OUT_CH], f32, name=f"y_ps_{s - DEPTH}", tag="y_ps"
                )
            # interleave tile s's transposes between tile (s-DEPTH)'s matmul
            # pairs: every LDWEIGHTS hides under the previous 512-col stream
            for c in range(KCH):
                if s < n:
                    nc.tensor.transpose(
                        xt_ps[:, c * 128 : (c + 1) * 128],
                        xh[:, c * 128 : (c + 1) * 128],
                        identity,
                    )
                if s >= DEPTH:
                    for half in range(2):
                        nc.tensor.matmul(
                            y_ps[:, half * 512 : (half + 1) * 512],
                            lhsT=xt[:, c * 128 : (c + 1) * 128],
                            rhs=wt_sb[c][:, half * 512 : (half + 1) * 512],
                            start=(c == 0),
                            stop=(c == KCH - 1),
                        )
            if s < n:
                xt_new = xt_sb_pool.tile([128, IN_CH], bf16, name=f"xt_{s}", tag="xt")
                nc.vector.tensor_copy(xt_new, xt_ps)
                xts[s] = xt_new
            if s >= DEPTH:
                t = s - DEPTH
                y_sb = y_sb_pool.tile([128, OUT_CH], bf16, name=f"y_sb_{t}", tag="y_sb")
                nc.scalar.copy(y_sb[:, 0:512], y_ps[:, 0:512])
                nc.vector.tensor_copy(y_sb[:, 512:1024], y_ps[:, 512:1024])
                # y stores ride the scalar HWDGE queue (free after the wt
                # load) so they never sit behind x loads on the sync queue
                nc.scalar.dma_start(y[t * 128 : (t + 1) * 128, :], y_sb)

    nc.compile()
    return nc


def get_nc(tok_per_core=TOK_PER_CORE):
    if tok_per_core not in _CACHE:
        _CACHE[tok_per_core] = build_nc(tok_per_core)
    return _CACHE[tok_per_core]


def _build_wt_bf16(w: np.ndarray) -> np.ndarray:
    """Dense W.T [in, out] in bf16 from compressed w[R, Q, BS] (host side,
    same construction as the reference's _build_dense_weight)."""
    import ml_dtypes

    i = np.arange(BS)
    idx = (i[:, None] - i[None, :]) % BS          # (bs, bs) circulant index
    Wb = w[:, :, idx]                             # (R, Q, bs, bs)
    W = Wb.transpose(0, 2, 1, 3).reshape(OUT_CH, IN_CH)
    return np.ascontiguousarray(W.T).astype(ml_dtypes.bfloat16)


def kernel(x: np.ndarray, w: np.ndarray) -> np.ndarray:
    import ml_dtypes

    from concourse.bass_utils import run_bass_kernel_spmd

    x = np.ascontiguousarray(x, dtype=np.float32)
    w = np.ascontiguousarray(w, dtype=np.float32)
    assert x.shape == (N_TOKENS, IN_CH), x.shape
    assert w.shape == (R, Q, BS), w.shape

    wt = _build_wt_bf16(w)
    ident = np.eye(128, dtype=ml_dtypes.bfloat16)
    nc = get_nc()
    in_maps = [
        {"x": x[i * TOK_PER_CORE : (i + 1) * TOK_PER_CORE], "wt": wt, "ident": ident}
        for i in range(N_CORES)
    ]
    res = run_bass_kernel_spmd(nc, in_maps, core_ids=list(range(N_CORES)))
    return np.concatenate(
        [np.asarray(r["y"]).astype(np.float32) for r in res.results], axis=0
    )
